# revision 5
# baseline (speedup 1.0000x reference)
"""Trainium2 Bass kernel for nn_AttentionBlock (B=2, S=4096, HID=256, 8 heads).

Sharding: 8 cores = 2 batches x 4 query-chunks of 1024 queries.
Each core redundantly computes full K/V projections for its batch, then
attention for its 1024 queries over all 8 heads, then the output projection.
Host gathers by concatenation (no cross-core reduction needed).

Device pipeline per core (bf16 matmul datapath, fp32 PSUM accumulation,
fp32 softmax normalization and output projection):
  - cast input tiles to bf16, PE-transpose, project -> qT/kT in [c, s]
    layout (head-major partitions).
  - Scores computed transposed: sT[k, q] via 4-way row-packed K=32 bf16
    matmuls (tile_position row strips), 512 queries per matmul.
  - exp on ACT with per-partition mask bias (mask folded to {0, -1e4}) and
    the 1/sqrt(32) scale folded into the activation scale; bf16 output.
  - PV bf16 matmuls col-packed in pairs with a ones-augmented V column per
    head, so the softmax denominator accumulates alongside weightedT in
    fp32 PSUM.
  - Normalization (fp32): reciprocal of denominator rows + K=1 broadcast
    matmuls, then elementwise multiply.
  - Output projection (fp32) from the stacked weightedT layout against a
    zero-padded Wo arrangement; bias added via a K=1 ones matmul.
"""

import numpy as np

import concourse.bacc as bacc
import concourse.bass as bass
from concourse import mybir
from concourse.tile import TileContext
from concourse.masks import make_identity
from concourse.bass_utils import run_bass_kernel_spmd

F32 = mybir.dt.float32
BF16 = mybir.dt.bfloat16
I32 = mybir.dt.int32
AF = mybir.ActivationFunctionType

HID = 256
HEADS = 8
DH = 32
SK = 4096
SQ = 1024  # queries per core
SCALE = 1.0 / np.sqrt(32.0)
NEG = -10000.0

_CACHE = {}


def _build_nc():
    nc = bacc.Bacc("TRN2", target_bir_lowering=False, debug=False, num_devices=8)

    q_d = nc.dram_tensor("q_in", [SQ, HID], F32, kind="ExternalInput").ap()
    k_d = nc.dram_tensor("k_in", [SK, HID], F32, kind="ExternalInput").ap()
    v_d = nc.dram_tensor("v_in", [SK, HID], F32, kind="ExternalInput").ap()
    m_d = nc.dram_tensor("mask_in", [128, 32], I32, kind="ExternalInput").ap()
    wq_d = nc.dram_tensor("wq", [HID, HID], F32, kind="ExternalInput").ap()
    wk_d = nc.dram_tensor("wk", [HID, HID], F32, kind="ExternalInput").ap()
    wv_d = nc.dram_tensor("wv", [HID, HID], F32, kind="ExternalInput").ap()
    wo_d = nc.dram_tensor("wo_arr", [128, 1024], F32, kind="ExternalInput").ap()
    bq_d = nc.dram_tensor("bq2", [128, 2], F32, kind="ExternalInput").ap()
    bk_d = nc.dram_tensor("bk2", [128, 2], F32, kind="ExternalInput").ap()
    bo_d = nc.dram_tensor("bo2", [1, HID], F32, kind="ExternalInput").ap()
    out_d = nc.dram_tensor("out", [SQ, HID], F32, kind="ExternalOutput").ap()

    from contextlib import ExitStack

    with TileContext(nc) as tc, ExitStack() as top:
        const = top.enter_context(tc.tile_pool(name="const", bufs=1))
        persist = top.enter_context(tc.tile_pool(name="persist", bufs=1))
        io_pool = top.enter_context(tc.tile_pool(name="io", bufs=6))
        vt_pool = top.enter_context(tc.tile_pool(name="vt", bufs=3))
        pt_pool = top.enter_context(tc.tile_pool(name="pt", bufs=3))
        rc_pool = top.enter_context(tc.tile_pool(name="rc", bufs=2))
        wtn_pool = top.enter_context(tc.tile_pool(name="wtn", bufs=4))
        osb_pool = top.enter_context(tc.tile_pool(name="osb", bufs=4))

        # ---------------- constants ----------------
        wq_bf = []
        wk_bf = []
        wv_bf = []
        for t in range(2):
            for nm, d_ap, lst in (("wq", wq_d, wq_bf), ("wk", wk_d, wk_bf),
                                  ("wv", wv_d, wv_bf)):
                wf = const.tile([128, 256], F32, name=f"{nm}_f{t}")
                nc.sync.dma_start(wf, d_ap[t * 128:(t + 1) * 128, :])
                wb = const.tile([128, 256], BF16, name=f"{nm}_b{t}")
                nc.vector.tensor_copy(wb, wf)
                lst.append(wb)
        wo_sb = const.tile([128, 1024], F32, name="wo_sb")
        nc.sync.dma_start(wo_sb, wo_d)
        bq_sb = const.tile([128, 2], F32, name="bq_sb")
        nc.sync.dma_start(bq_sb, bq_d)
        bk_sb = const.tile([128, 2], F32, name="bk_sb")
        nc.sync.dma_start(bk_sb, bk_d)
        bo_sb = const.tile([1, HID], F32, name="bo_sb")
        nc.sync.dma_start(bo_sb, bo_d)

        ident_bf = const.tile([128, 128], BF16, name="ident_bf")
        make_identity(nc, ident_bf)
        ones_sb = const.tile([128, 128], F32, name="ones_sb")
        nc.vector.memset(ones_sb, 1.0)

        mask_i = const.tile([128, 32], I32, name="mask_i")
        nc.sync.dma_start(mask_i, m_d)
        mask_f = const.tile([128, 32], F32, name="mask_f")
        nc.vector.tensor_copy(mask_f, mask_i)
        maskbias = const.tile([128, 32], F32, name="maskbias")
        # (m - 1) * 1e4  ->  0 for keep, -1e4 for masked
        nc.vector.tensor_scalar(maskbias, mask_f, -NEG, NEG,
                                op0=mybir.AluOpType.mult,
                                op1=mybir.AluOpType.add)

        # ---------------- persistent buffers ----------------
        qT_sb = [persist.tile([128, SQ], BF16, name=f"qT_sb{g}") for g in range(2)]
        kT_sb = [persist.tile([128, SK], BF16, name=f"kT_sb{g}") for g in range(2)]
        qTin = [persist.tile([128, SQ], BF16, name=f"qTin{t}") for t in range(2)]
        kTin = [persist.tile([128, SK], BF16, name=f"kTin{t}") for t in range(2)]
        vaug = persist.tile([128, 32 * 264], BF16, name="vaug")
        nc.vector.memset(vaug, 1.0)

        # ---------------- phase A: transposes + projections ----------------
        with ExitStack() as pa:
            tpsum = pa.enter_context(tc.tile_pool(name="tpsum", bufs=3, space="PSUM"))
            ppsum = pa.enter_context(tc.tile_pool(name="ppsum", bufs=3, space="PSUM"))

            for src_d, dstT, nst in ((q_d, qTin, SQ // 128), (k_d, kTin, SK // 128)):
                for sg in range(nst // 4):
                    tp = [tpsum.tile([128, 512], BF16, tag="tp", name=f"tp{t}")
                          for t in range(2)]
                    for j in range(4):
                        s = sg * 4 + j
                        xin = io_pool.tile([128, 256], F32, tag="xin", name="xin")
                        nc.sync.dma_start(xin, src_d[s * 128:(s + 1) * 128, :])
                        xbf = io_pool.tile([128, 256], BF16, tag="xbf", name="xbf")
                        nc.vector.tensor_copy(xbf, xin)
                        for t in range(2):
                            nc.tensor.transpose(
                                tp[t][:, j * 128:(j + 1) * 128],
                                xbf[:, t * 128:(t + 1) * 128], ident_bf)
                    for t in range(2):
                        nc.vector.tensor_copy(
                            dstT[t][:, sg * 512:(sg + 1) * 512], tp[t])

            for xT, w_bf, b_sb, outT, slen in (
                    (qTin, wq_bf, bq_sb, qT_sb, SQ),
                    (kTin, wk_bf, bk_sb, kT_sb, SK)):
                for g in range(2):
                    for nch in range(slen // 512):
                        ps = ppsum.tile([128, 512], F32, tag="proj", name="ps")
                        for t in range(2):
                            nc.tensor.matmul(
                                ps,
                                w_bf[t][:, g * 128:(g + 1) * 128],
                                xT[t][:, nch * 512:(nch + 1) * 512],
                                start=(t == 0), stop=(t == 1))
                        nc.vector.tensor_scalar_add(
                            outT[g][:, nch * 512:(nch + 1) * 512],
                            ps, b_sb[:, g:g + 1])

            # value: transpose -> project -> strided-scatter into vaug
            for s in range(SK // 128):
                vin = io_pool.tile([128, 256], F32, tag="xin", name="vin")
                nc.sync.dma_start(vin, v_d[s * 128:(s + 1) * 128, :])
                vbf = io_pool.tile([128, 256], BF16, tag="xbf", name="vbf")
                nc.vector.tensor_copy(vbf, vin)
                vtp = tpsum.tile([128, 256], BF16, tag="tp", name="vtp")
                for t in range(2):
                    nc.tensor.transpose(
                        vtp[:, t * 128:(t + 1) * 128],
                        vbf[:, t * 128:(t + 1) * 128], ident_bf)
                vT = vt_pool.tile([128, 256], BF16, tag="vT", name="vT")
                nc.vector.tensor_copy(vT, vtp)
                vps = ppsum.tile([128, 256], F32, tag="proj", name="vps")
                for t in range(2):
                    nc.tensor.matmul(
                        vps, vT[:, t * 128:(t + 1) * 128], wv_bf[t],
                        start=(t == 0), stop=(t == 1))
                dst = vaug[:, s * 264:(s + 1) * 264]
                dst = dst.rearrange("p (h e) -> p h e", e=33)[:, :, 0:DH]
                src = vps.rearrange("p (h e) -> p h e", e=DH)
                nc.vector.tensor_copy(dst, src)

        # ---------------- phase B/C: attention ----------------
        with ExitStack() as pb:
            st_pool = pb.enter_context(tc.tile_pool(name="stp", bufs=2, space="PSUM"))
            wt_pool = pb.enter_context(tc.tile_pool(name="wtp", bufs=4, space="PSUM"))

            for qc in range(SQ // 512):
                wts = [wt_pool.tile([128, 512], F32, tag="wt", name=f"wt{p}")
                       for p in range(4)]
                for g in range(2):
                    for kt in range(SK // 128):
                        for jj in range(2):
                            p = 2 * g + jj
                            st = st_pool.tile([128, 1024], F32, tag="st", name="st")
                            for j2 in range(2):
                                j = 2 * jj + j2
                                nc.tensor.matmul(
                                    st[:, j2 * 512:(j2 + 1) * 512],
                                    kT_sb[g][32 * j:32 * j + 32,
                                             kt * 128:(kt + 1) * 128],
                                    qT_sb[g][32 * j:32 * j + 32,
                                             qc * 512:(qc + 1) * 512],
                                    start=True, stop=True,
                                    tile_position=(32 * j, 0))
                            ptile = pt_pool.tile([128, 1024], BF16, tag="pt",
                                                 name="ptile")
                            nc.scalar.activation(
                                ptile, st, AF.Exp,
                                bias=maskbias[:, kt:kt + 1], scale=SCALE)
                            for j2 in range(2):
                                h = 4 * g + 2 * jj + j2
                                nc.tensor.matmul(
                                    wts[p][64 * j2:64 * j2 + 33, :],
                                    vaug[:, kt * 264 + 33 * h:
                                         kt * 264 + 33 * h + 33],
                                    ptile[:, j2 * 512:(j2 + 1) * 512],
                                    start=(kt == 0), stop=(kt == SK // 128 - 1),
                                    tile_position=(0, 64 * j2),
                                    skip_group_check=True)

                # normalization + output projection for this query chunk
                wtns = []
                for p in range(4):
                    rc = rc_pool.tile([128, 512], F32, tag="rc", name="rc")
                    nc.vector.reciprocal(rc[32:33, :], wts[p][32:33, :])
                    nc.vector.reciprocal(rc[96:97, :], wts[p][96:97, :])
                    bc = st_pool.tile([128, 512], F32, tag="st", name="bc")
                    nc.tensor.matmul(bc[0:32, :], ones_sb[32:33, 0:32],
                                     rc[32:33, :], start=True, stop=True,
                                     tile_position=(32, 0))
                    nc.tensor.matmul(bc[64:96, :], ones_sb[96:97, 0:32],
                                     rc[96:97, :], start=True, stop=True,
                                     tile_position=(96, 64))
                    bcs = osb_pool.tile([128, 512], F32, tag="bcs", name="bcs")
                    nc.vector.tensor_copy(bcs[0:32, :], bc[0:32, :])
                    nc.vector.tensor_copy(bcs[64:96, :], bc[64:96, :])
                    wtn = wtn_pool.tile([128, 512], F32, tag="wtn", name="wtn")
                    nc.vector.memset(wtn, 0.0)
                    nc.vector.tensor_mul(wtn[0:32, :], wts[p][0:32, :],
                                         bcs[0:32, :])
                    nc.vector.tensor_mul(wtn[64:96, :], wts[p][64:96, :],
                                         bcs[64:96, :])
                    wtns.append(wtn)

                for m in range(4):
                    ops = wt_pool.tile([128, 256], F32, tag="wt", name="ops")
                    for p in range(4):
                        nc.tensor.matmul(
                            ops, wtns[p][:, m * 128:(m + 1) * 128],
                            wo_sb[:, p * 256:(p + 1) * 256],
                            start=(p == 0), stop=False,
                            skip_group_check=True)
                    nc.tensor.matmul(ops, ones_sb[0:1, 0:128], bo_sb,
                                     start=False, stop=True,
                                     skip_group_check=True)
                    ob = osb_pool.tile([128, 256], F32, tag="ob", name="ob")
                    nc.vector.tensor_copy(ob, ops)
                    nc.sync.dma_start(
                        out_d[qc * 512 + m * 128:qc * 512 + (m + 1) * 128, :],
                        ob)

    nc.finalize()
    return nc


def _get_nc():
    if "nc" not in _CACHE:
        _CACHE["nc"] = _build_nc()
    return _CACHE["nc"]


def kernel(query, key, value, mask, Wq, bq, Wk, bk, Wv, bv, Wo, bo,
           _trace=False):
    query = np.asarray(query, np.float32)
    key = np.asarray(key, np.float32)
    value = np.asarray(value, np.float32)
    mask = np.asarray(mask, np.int32)
    Wq = np.ascontiguousarray(np.asarray(Wq, np.float32))
    Wk = np.ascontiguousarray(np.asarray(Wk, np.float32))
    Wv = np.ascontiguousarray(np.asarray(Wv, np.float32))
    Wo = np.ascontiguousarray(np.asarray(Wo, np.float32))
    bq = np.asarray(bq, np.float32)
    bk = np.asarray(bk, np.float32)
    bv = np.asarray(bv, np.float32)
    bo = np.asarray(bo, np.float32)

    nc = _get_nc()

    wo_arr = np.zeros((128, 4, 256), np.float32)
    for p in range(4):
        wo_arr[0:32, p] = Wo[64 * p:64 * p + 32]
        wo_arr[64:96, p] = Wo[64 * p + 32:64 * p + 64]
    wo_arr = np.ascontiguousarray(wo_arr.reshape(128, 1024))
    bq2 = np.ascontiguousarray(bq.reshape(2, 128).T)
    bk2 = np.ascontiguousarray(bk.reshape(2, 128).T)
    bo2 = np.ascontiguousarray((bv @ Wo + bo).reshape(1, 256))

    in_maps = []
    for c in range(8):
        b, qi = divmod(c, 4)
        mb = np.ascontiguousarray(mask[b, 0].reshape(32, 128).T)
        in_maps.append({
            "q_in": np.ascontiguousarray(query[b, qi * SQ:(qi + 1) * SQ]),
            "k_in": np.ascontiguousarray(key[b]),
            "v_in": np.ascontiguousarray(value[b]),
            "mask_in": mb,
            "wq": Wq, "wk": Wk, "wv": Wv, "wo_arr": wo_arr,
            "bq2": bq2, "bk2": bk2, "bo2": bo2,
        })

    res = run_bass_kernel_spmd(nc, in_maps, core_ids=list(range(8)),
                               trace=_trace)
    if _trace:
        _CACHE["last_result"] = res

    out = np.empty((2, 4096, HID), np.float32)
    for c in range(8):
        b, qi = divmod(c, 4)
        out[b, qi * SQ:(qi + 1) * SQ] = res.results[c]["out"]
    return out


# revision 7
# speedup vs baseline: 1.0549x; 1.0549x over previous
"""Trainium2 Bass kernel for nn_AttentionBlock (B=2, S=4096, HID=256, 8 heads).

Sharding: 8 cores = 2 batches x 4 query-chunks of 1024 queries.
Each core redundantly computes full K/V projections for its batch, then
attention for its 1024 queries over all 8 heads, then the output projection.
Host gathers by concatenation (no cross-core reduction needed).

Device pipeline per core (fp16 matmul datapath, fp32 PSUM accumulation,
fp32 softmax normalization and output projection):
  - PE-transpose fp32 input tiles, cast to fp16 during PSUM eviction,
    project -> qT/kT in [c, s] layout (head-major partitions). kT and the
    augmented V are produced in chunks so attention starts early.
  - Scores computed transposed: sT[k, q] via 4-way row-packed K=32 fp16
    matmuls (tile_position row strips), 512 queries per matmul.
  - exp on ACT with per-partition mask bias (mask folded to {0, -1e4}) and
    the 1/sqrt(32) scale folded into the activation scale; fp16 output.
  - PV fp16 matmuls col-packed in pairs with a ones-augmented V column per
    head, so the softmax denominator accumulates alongside weightedT in
    fp32 PSUM.
  - Normalization (fp32): reciprocal of denominator rows + K=1 broadcast
    matmuls, then elementwise multiply.
  - Output projection (fp32) from the stacked weightedT layout against a
    zero-padded Wo arrangement; bias added via a K=1 ones matmul.
"""

import numpy as np

import concourse.bacc as bacc
import concourse.bass as bass
from concourse import mybir
from concourse.tile import TileContext
from concourse.masks import make_identity
from concourse.bass_utils import run_bass_kernel_spmd

F32 = mybir.dt.float32
F16 = mybir.dt.float16
I32 = mybir.dt.int32
AF = mybir.ActivationFunctionType

HID = 256
HEADS = 8
DH = 32
SK = 4096
SQ = 1024  # queries per core
NKT = SK // 128  # 32 key tiles
SCALE = 1.0 / np.sqrt(32.0)
NEG = -10000.0

_CACHE = {}


def _build_nc():
    nc = bacc.Bacc("TRN2", target_bir_lowering=False, debug=False, num_devices=8)

    q_d = nc.dram_tensor("q_in", [SQ, HID], F32, kind="ExternalInput").ap()
    k_d = nc.dram_tensor("k_in", [SK, HID], F32, kind="ExternalInput").ap()
    v_d = nc.dram_tensor("v_in", [SK, HID], F32, kind="ExternalInput").ap()
    m_d = nc.dram_tensor("mask_in", [128, 32], I32, kind="ExternalInput").ap()
    wq_d = nc.dram_tensor("wq", [HID, HID], F32, kind="ExternalInput").ap()
    wk_d = nc.dram_tensor("wk", [HID, HID], F32, kind="ExternalInput").ap()
    wv_d = nc.dram_tensor("wv", [HID, HID], F32, kind="ExternalInput").ap()
    wo_d = nc.dram_tensor("wo_arr", [128, 1024], F32, kind="ExternalInput").ap()
    bq_d = nc.dram_tensor("bq2", [128, 2], F32, kind="ExternalInput").ap()
    bk_d = nc.dram_tensor("bk2", [128, 2], F32, kind="ExternalInput").ap()
    bo_d = nc.dram_tensor("bo2", [1, HID], F32, kind="ExternalInput").ap()
    out_d = nc.dram_tensor("out", [SQ, HID], F32, kind="ExternalOutput").ap()

    from contextlib import ExitStack

    with TileContext(nc) as tc, ExitStack() as top:
        const = top.enter_context(tc.tile_pool(name="const", bufs=1))
        persist = top.enter_context(tc.tile_pool(name="persist", bufs=1))
        io_pool = top.enter_context(tc.tile_pool(name="io", bufs=8))
        xt_pool = top.enter_context(tc.tile_pool(name="xt", bufs=3))
        vt_pool = top.enter_context(tc.tile_pool(name="vt", bufs=3))
        pt_pool = top.enter_context(tc.tile_pool(name="pt", bufs=4))
        rc_pool = top.enter_context(tc.tile_pool(name="rc", bufs=2))
        wtn_pool = top.enter_context(tc.tile_pool(name="wtn", bufs=5))
        osb_pool = top.enter_context(tc.tile_pool(name="osb", bufs=4))

        tpsum = top.enter_context(tc.tile_pool(name="tpsum", bufs=1, space="PSUM"))
        ppsum = top.enter_context(tc.tile_pool(name="ppsum", bufs=1, space="PSUM"))
        st_pool = top.enter_context(tc.tile_pool(name="stp", bufs=2, space="PSUM"))
        wt_pool = top.enter_context(tc.tile_pool(name="wtp", bufs=2, space="PSUM"))

        # ---------------- constants ----------------
        wq_hf = []
        wk_hf = []
        wv_hf = []
        for t in range(2):
            for nm, d_ap, lst in (("wq", wq_d, wq_hf), ("wk", wk_d, wk_hf),
                                  ("wv", wv_d, wv_hf)):
                wf = const.tile([128, 256], F32, name=f"{nm}_f{t}")
                nc.sync.dma_start(wf, d_ap[t * 128:(t + 1) * 128, :])
                wb = const.tile([128, 256], F16, name=f"{nm}_h{t}")
                nc.vector.tensor_copy(wb, wf)
                lst.append(wb)
        wo_sb = const.tile([128, 1024], F32, name="wo_sb")
        nc.sync.dma_start(wo_sb, wo_d)
        bq_sb = const.tile([128, 2], F32, name="bq_sb")
        nc.sync.dma_start(bq_sb, bq_d)
        bk_sb = const.tile([128, 2], F32, name="bk_sb")
        nc.sync.dma_start(bk_sb, bk_d)
        bo_sb = const.tile([1, HID], F32, name="bo_sb")
        nc.sync.dma_start(bo_sb, bo_d)

        identity = const.tile([128, 128], F32, name="identity")
        make_identity(nc, identity)
        ones_sb = const.tile([128, 128], F32, name="ones_sb")
        nc.vector.memset(ones_sb, 1.0)

        mask_i = const.tile([128, 32], I32, name="mask_i")
        nc.sync.dma_start(mask_i, m_d)
        mask_f = const.tile([128, 32], F32, name="mask_f")
        nc.vector.tensor_copy(mask_f, mask_i)
        maskbias = const.tile([128, 32], F32, name="maskbias")
        # (m - 1) * 1e4  ->  0 for keep, -1e4 for masked
        nc.vector.tensor_scalar(maskbias, mask_f, -NEG, NEG,
                                op0=mybir.AluOpType.mult,
                                op1=mybir.AluOpType.add)

        # ---------------- persistent buffers ----------------
        qT_sb = [persist.tile([128, SQ], F16, name=f"qT_sb{g}") for g in range(2)]
        # kT in 512-col chunks (4 ktiles each) so attention can start early
        kT_ch = [[persist.tile([128, 512], F16, name=f"kT{g}_{c}")
                  for c in range(SK // 512)] for g in range(2)]
        # augmented V, one [128, 264] tile per ktile (ones in column 33h+32)
        vaug = [persist.tile([128, 264], F16, name=f"vaug{s}")
                for s in range(NKT)]
        for s in range(NKT):
            nc.vector.memset(vaug[s], 1.0)

        # ---------------- phase A helpers ----------------
        def transpose_group(src_d, sg):
            """Load 4 s-tiles, return [xTf16 chunk tiles for ci-half 0/1]:
            each [128, 512] fp16, rows = ci half, cols = 512 seq."""
            xins = []
            for j in range(4):
                s = sg * 4 + j
                xin = io_pool.tile([128, 256], F32, tag="xin", name="xin")
                nc.sync.dma_start(xin, src_d[s * 128:(s + 1) * 128, :])
                xins.append(xin)
            chunks = []
            for t in range(2):
                tp = tpsum.tile([128, 512], F32, tag="tp", name="tp")
                for j in range(4):
                    nc.tensor.transpose(
                        tp[:, j * 128:(j + 1) * 128],
                        xins[j][:, t * 128:(t + 1) * 128], identity)
                ch = xt_pool.tile([128, 512], F16, tag="xch", name="xch")
                nc.vector.tensor_copy(ch, tp)
                chunks.append(ch)
            return chunks

        def project_chunk(chunks, w_hf, b_sb, outs):
            """outs[g] = [128, 512] destination AP for group g."""
            for g in range(2):
                ps = ppsum.tile([128, 512], F32, tag="proj", name="ps")
                for t in range(2):
                    nc.tensor.matmul(
                        ps, w_hf[t][:, g * 128:(g + 1) * 128], chunks[t],
                        start=(t == 0), stop=(t == 1))
                nc.vector.tensor_scalar_add(outs[g], ps, b_sb[:, g:g + 1])

        def value_tile(s):
            vin = io_pool.tile([128, 256], F32, tag="xin", name="vin")
            nc.sync.dma_start(vin, v_d[s * 128:(s + 1) * 128, :])
            vtp = tpsum.tile([128, 256], F32, tag="tp", name="vtp")
            for t in range(2):
                nc.tensor.transpose(
                    vtp[:, t * 128:(t + 1) * 128],
                    vin[:, t * 128:(t + 1) * 128], identity)
            vT = vt_pool.tile([128, 256], F16, tag="vT", name="vT")
            nc.vector.tensor_copy(vT, vtp)
            vps = ppsum.tile([128, 256], F32, tag="proj", name="vps")
            for t in range(2):
                nc.tensor.matmul(
                    vps, vT[:, t * 128:(t + 1) * 128], wv_hf[t],
                    start=(t == 0), stop=(t == 1))
            dst = vaug[s].rearrange("p (h e) -> p h e", e=33)[:, :, 0:DH]
            src = vps.rearrange("p (h e) -> p h e", e=DH)
            nc.vector.tensor_copy(dst, src)

        # ---------------- phase A emission (query, then k/v interleaved) ---
        for sg in range(SQ // 512):
            chunks = transpose_group(q_d, sg)
            project_chunk(chunks, wq_hf, bq_sb,
                          [qT_sb[g][:, sg * 512:(sg + 1) * 512]
                           for g in range(2)])
        for cch in range(SK // 512):
            chunks = transpose_group(k_d, cch)
            project_chunk(chunks, wk_hf, bk_sb,
                          [kT_ch[g][cch] for g in range(2)])
            for s in range(cch * 4, cch * 4 + 4):
                value_tile(s)

        # ---------------- phase B/C: attention ----------------
        for qc in range(SQ // 512):
            wtns = []
            for g in range(2):
                wts = [wt_pool.tile([128, 512], F32, tag="wt", name=f"wt{jj}")
                       for jj in range(2)]
                for kt in range(NKT):
                    for jj in range(2):
                        st = st_pool.tile([128, 1024], F32, tag="st", name="st")
                        for j2 in range(2):
                            j = 2 * jj + j2
                            nc.tensor.matmul(
                                st[:, j2 * 512:(j2 + 1) * 512],
                                kT_ch[g][kt // 4][32 * j:32 * j + 32,
                                                  (kt % 4) * 128:
                                                  (kt % 4) * 128 + 128],
                                qT_sb[g][32 * j:32 * j + 32,
                                         qc * 512:(qc + 1) * 512],
                                start=True, stop=True,
                                tile_position=(32 * j, 0))
                        ptile = pt_pool.tile([128, 1024], F16, tag="pt",
                                             name="ptile")
                        nc.scalar.activation(
                            ptile, st, AF.Exp,
                            bias=maskbias[:, kt:kt + 1], scale=SCALE)
                        for j2 in range(2):
                            h = 4 * g + 2 * jj + j2
                            nc.tensor.matmul(
                                wts[jj][64 * j2:64 * j2 + 33, :],
                                vaug[kt][:, 33 * h:33 * h + 33],
                                ptile[:, j2 * 512:(j2 + 1) * 512],
                                start=(kt == 0), stop=(kt == NKT - 1),
                                tile_position=(0, 64 * j2),
                                skip_group_check=True)

                # normalization for this (qc, g): frees the wt slots
                for jj in range(2):
                    rc = rc_pool.tile([128, 512], F32, tag="rc", name="rc")
                    nc.vector.reciprocal(rc[32:33, :], wts[jj][32:33, :])
                    nc.vector.reciprocal(rc[96:97, :], wts[jj][96:97, :])
                    bc = st_pool.tile([128, 512], F32, tag="st", name="bc")
                    nc.tensor.matmul(bc[0:32, :], ones_sb[32:33, 0:32],
                                     rc[32:33, :], start=True, stop=True,
                                     tile_position=(32, 0))
                    nc.tensor.matmul(bc[64:96, :], ones_sb[96:97, 0:32],
                                     rc[96:97, :], start=True, stop=True,
                                     tile_position=(96, 64))
                    bcs = osb_pool.tile([128, 512], F32, tag="bcs", name="bcs")
                    nc.vector.tensor_copy(bcs[0:32, :], bc[0:32, :])
                    nc.vector.tensor_copy(bcs[64:96, :], bc[64:96, :])
                    wtn = wtn_pool.tile([128, 512], F32, tag="wtn", name="wtn")
                    nc.vector.memset(wtn, 0.0)
                    nc.vector.tensor_mul(wtn[0:32, :], wts[jj][0:32, :],
                                         bcs[0:32, :])
                    nc.vector.tensor_mul(wtn[64:96, :], wts[jj][64:96, :],
                                         bcs[64:96, :])
                    wtns.append(wtn)

            for m in range(4):
                ops = wt_pool.tile([128, 256], F32, tag="wt", name="ops")
                for p in range(4):
                    nc.tensor.matmul(
                        ops, wtns[p][:, m * 128:(m + 1) * 128],
                        wo_sb[:, p * 256:(p + 1) * 256],
                        start=(p == 0), stop=False,
                        skip_group_check=True)
                nc.tensor.matmul(ops, ones_sb[0:1, 0:128], bo_sb,
                                 start=False, stop=True,
                                 skip_group_check=True)
                ob = osb_pool.tile([128, 256], F32, tag="ob", name="ob")
                nc.vector.tensor_copy(ob, ops)
                nc.sync.dma_start(
                    out_d[qc * 512 + m * 128:qc * 512 + (m + 1) * 128, :],
                    ob)

    nc.finalize()
    return nc


def _get_nc():
    if "nc" not in _CACHE:
        _CACHE["nc"] = _build_nc()
    return _CACHE["nc"]


def kernel(query, key, value, mask, Wq, bq, Wk, bk, Wv, bv, Wo, bo,
           _trace=False):
    query = np.asarray(query, np.float32)
    key = np.asarray(key, np.float32)
    value = np.asarray(value, np.float32)
    mask = np.asarray(mask, np.int32)
    Wq = np.ascontiguousarray(np.asarray(Wq, np.float32))
    Wk = np.ascontiguousarray(np.asarray(Wk, np.float32))
    Wv = np.ascontiguousarray(np.asarray(Wv, np.float32))
    Wo = np.ascontiguousarray(np.asarray(Wo, np.float32))
    bq = np.asarray(bq, np.float32)
    bk = np.asarray(bk, np.float32)
    bv = np.asarray(bv, np.float32)
    bo = np.asarray(bo, np.float32)

    nc = _get_nc()

    wo_arr = np.zeros((128, 4, 256), np.float32)
    for p in range(4):
        wo_arr[0:32, p] = Wo[64 * p:64 * p + 32]
        wo_arr[64:96, p] = Wo[64 * p + 32:64 * p + 64]
    wo_arr = np.ascontiguousarray(wo_arr.reshape(128, 1024))
    bq2 = np.ascontiguousarray(bq.reshape(2, 128).T)
    bk2 = np.ascontiguousarray(bk.reshape(2, 128).T)
    bo2 = np.ascontiguousarray((bv @ Wo + bo).reshape(1, 256))

    in_maps = []
    for c in range(8):
        b, qi = divmod(c, 4)
        mb = np.ascontiguousarray(mask[b, 0].reshape(32, 128).T)
        in_maps.append({
            "q_in": np.ascontiguousarray(query[b, qi * SQ:(qi + 1) * SQ]),
            "k_in": np.ascontiguousarray(key[b]),
            "v_in": np.ascontiguousarray(value[b]),
            "mask_in": mb,
            "wq": Wq, "wk": Wk, "wv": Wv, "wo_arr": wo_arr,
            "bq2": bq2, "bk2": bk2, "bo2": bo2,
        })

    res = run_bass_kernel_spmd(nc, in_maps, core_ids=list(range(8)),
                               trace=_trace)
    if _trace:
        _CACHE["last_result"] = res

    out = np.empty((2, 4096, HID), np.float32)
    for c in range(8):
        b, qi = divmod(c, 4)
        out[b, qi * SQ:(qi + 1) * SQ] = res.results[c]["out"]
    return out


# revision 8
# speedup vs baseline: 1.6158x; 1.5316x over previous
"""Trainium2 Bass kernel for nn_AttentionBlock (B=2, S=4096, HID=256, 8 heads).

Sharding: 8 cores = 2 batches x 4 query-chunks of 1024 queries.
Each core redundantly computes full K/V projections for its batch, then
attention for its 1024 queries over all 8 heads, then the output projection.
Host gathers by concatenation (no cross-core reduction needed).

Mask compaction: the (b,1,S) key mask zeroes whole keys for every query and
head, so masked keys are dropped up front. The host computes the surviving
key indices (padded with a duplicate index to a multiple of 512; padding
slots carry mask=0 so they still contribute exactly zero), and the device
gathers those key/value rows from DRAM via indirect DMA. Attention then
runs over the compacted key set only.

Device pipeline per core (fp16 matmul datapath, fp32 PSUM accumulation,
fp32 softmax normalization):
  - PE-transpose fp32 input tiles, cast to fp16 during PSUM eviction,
    project -> qT/kT in [c, s] layout (head-major partitions). kT and the
    augmented V are produced in chunks so attention starts early.
  - Scores computed transposed: sT[k, q] via 4-way row-packed K=32 fp16
    matmuls (tile_position row strips), 512 queries per matmul.
  - exp on ACT with per-partition mask bias (mask folded to {0, -1e4}) and
    the 1/sqrt(32) scale folded into the activation scale; fp16 output.
  - PV fp16 matmuls col-packed in pairs with a ones-augmented V column per
    head, so the softmax denominator accumulates alongside weightedT in
    fp32 PSUM.
  - Normalization: weightedT evicted to SBUF immediately (frees the PSUM
    accumulators), then reciprocal of denominator rows + K=1 broadcast
    matmuls + elementwise multiply, off the critical path.
  - Output projection (fp16 operands, fp32 accumulate) from the stacked
    weightedT layout against a zero-padded Wo arrangement; bias added via
    a K=1 ones matmul.
"""

import numpy as np

import concourse.bacc as bacc
import concourse.bass as bass
from concourse import mybir
from concourse.tile import TileContext
from concourse.masks import make_identity
from concourse.bass_utils import run_bass_kernel_spmd

F32 = mybir.dt.float32
F16 = mybir.dt.float16
I32 = mybir.dt.int32
AF = mybir.ActivationFunctionType

HID = 256
HEADS = 8
DH = 32
SK = 4096
SQ = 1024  # queries per core
SCALE = 1.0 / np.sqrt(32.0)
NEG = -10000.0

_CACHE = {}


def _build_nc(nkc):
    """nkc = number of 128-key tiles after mask compaction (multiple of 4)."""
    skc = nkc * 128
    nc = bacc.Bacc("TRN2", target_bir_lowering=False, debug=False, num_devices=8)

    q_d = nc.dram_tensor("q_in", [SQ, HID], F32, kind="ExternalInput").ap()
    k_d = nc.dram_tensor("k_in", [SK, HID], F32, kind="ExternalInput").ap()
    v_d = nc.dram_tensor("v_in", [SK, HID], F32, kind="ExternalInput").ap()
    m_d = nc.dram_tensor("mask_in", [128, nkc], I32, kind="ExternalInput").ap()
    i_d = nc.dram_tensor("idx_in", [128, nkc], I32, kind="ExternalInput").ap()
    wq_d = nc.dram_tensor("wq", [HID, HID], F32, kind="ExternalInput").ap()
    wk_d = nc.dram_tensor("wk", [HID, HID], F32, kind="ExternalInput").ap()
    wv_d = nc.dram_tensor("wv", [HID, HID], F32, kind="ExternalInput").ap()
    wo_d = nc.dram_tensor("wo_arr", [128, 1024], F32, kind="ExternalInput").ap()
    bq_d = nc.dram_tensor("bq2", [128, 2], F32, kind="ExternalInput").ap()
    bk_d = nc.dram_tensor("bk2", [128, 2], F32, kind="ExternalInput").ap()
    bo_d = nc.dram_tensor("bo2", [1, HID], F32, kind="ExternalInput").ap()
    out_d = nc.dram_tensor("out", [SQ, HID], F32, kind="ExternalOutput").ap()

    from contextlib import ExitStack

    with TileContext(nc) as tc, ExitStack() as top:
        const = top.enter_context(tc.tile_pool(name="const", bufs=1))
        persist = top.enter_context(tc.tile_pool(name="persist", bufs=1))
        io_pool = top.enter_context(tc.tile_pool(name="io", bufs=8))
        xt_pool = top.enter_context(tc.tile_pool(name="xt", bufs=3))
        vt_pool = top.enter_context(tc.tile_pool(name="vt", bufs=3))
        pt_pool = top.enter_context(tc.tile_pool(name="pt", bufs=4))
        rc_pool = top.enter_context(tc.tile_pool(name="rc", bufs=2))
        wtn_pool = top.enter_context(tc.tile_pool(name="wtn", bufs=5))
        osb_pool = top.enter_context(tc.tile_pool(name="osb", bufs=4))

        tpsum = top.enter_context(tc.tile_pool(name="tpsum", bufs=1, space="PSUM"))
        ppsum = top.enter_context(tc.tile_pool(name="ppsum", bufs=1, space="PSUM"))
        st_pool = top.enter_context(tc.tile_pool(name="stp", bufs=2, space="PSUM"))
        wt_pool = top.enter_context(tc.tile_pool(name="wtp", bufs=2, space="PSUM"))

        # ---------------- constants ----------------
        wq_hf = []
        wk_hf = []
        wv_hf = []
        for t in range(2):
            for nm, d_ap, lst in (("wq", wq_d, wq_hf), ("wk", wk_d, wk_hf),
                                  ("wv", wv_d, wv_hf)):
                wf = const.tile([128, 256], F32, name=f"{nm}_f{t}")
                nc.sync.dma_start(wf, d_ap[t * 128:(t + 1) * 128, :])
                wb = const.tile([128, 256], F16, name=f"{nm}_h{t}")
                nc.vector.tensor_copy(wb, wf)
                lst.append(wb)
        wo_f = const.tile([128, 1024], F32, name="wo_f")
        nc.sync.dma_start(wo_f, wo_d)
        wo_hf = const.tile([128, 1024], F16, name="wo_hf")
        nc.vector.tensor_copy(wo_hf, wo_f)
        bq_sb = const.tile([128, 2], F32, name="bq_sb")
        nc.sync.dma_start(bq_sb, bq_d)
        bk_sb = const.tile([128, 2], F32, name="bk_sb")
        nc.sync.dma_start(bk_sb, bk_d)
        bo_f = const.tile([1, HID], F32, name="bo_f")
        nc.sync.dma_start(bo_f, bo_d)
        bo_hf = const.tile([1, HID], F16, name="bo_hf")
        nc.vector.tensor_copy(bo_hf, bo_f)

        identity = const.tile([128, 128], F32, name="identity")
        make_identity(nc, identity)
        ones_sb = const.tile([128, 128], F32, name="ones_sb")
        nc.vector.memset(ones_sb, 1.0)
        ones_hf = const.tile([1, 128], F16, name="ones_hf")
        nc.vector.memset(ones_hf, 1.0)

        mask_i = const.tile([128, nkc], I32, name="mask_i")
        nc.sync.dma_start(mask_i, m_d)
        mask_f = const.tile([128, nkc], F32, name="mask_f")
        nc.vector.tensor_copy(mask_f, mask_i)
        maskbias = const.tile([128, nkc], F32, name="maskbias")
        # (m - 1) * 1e4  ->  0 for keep, -1e4 for masked/padded
        nc.vector.tensor_scalar(maskbias, mask_f, -NEG, NEG,
                                op0=mybir.AluOpType.mult,
                                op1=mybir.AluOpType.add)
        idx_sb = const.tile([128, nkc], I32, name="idx_sb")
        nc.sync.dma_start(idx_sb, i_d)

        # ---------------- persistent buffers ----------------
        qT_sb = [persist.tile([128, SQ], F16, name=f"qT_sb{g}") for g in range(2)]
        # kT in 512-col chunks (4 ktiles each) so attention can start early
        kT_ch = [[persist.tile([128, 512], F16, name=f"kT{g}_{c}")
                  for c in range(skc // 512)] for g in range(2)]
        # augmented V, one [128, 264] tile per ktile (ones in column 33h+32)
        vaug = [persist.tile([128, 264], F16, name=f"vaug{s}")
                for s in range(nkc)]
        for s in range(nkc):
            nc.vector.memset(vaug[s], 1.0)

        # ---------------- phase A helpers ----------------
        def load_tile(src_d, s, gather):
            xin = io_pool.tile([128, 256], F32, tag="xin", name="xin")
            if gather:
                nc.gpsimd.indirect_dma_start(
                    out=xin, out_offset=None, in_=src_d,
                    in_offset=bass.IndirectOffsetOnAxis(
                        ap=idx_sb[:, s:s + 1], axis=0))
            else:
                nc.sync.dma_start(xin, src_d[s * 128:(s + 1) * 128, :])
            return xin

        def transpose_group(src_d, sg, gather):
            """Load 4 s-tiles; return fp16 [128, 512] chunk per ci-half."""
            xins = [load_tile(src_d, sg * 4 + j, gather) for j in range(4)]
            chunks = []
            for t in range(2):
                tp = tpsum.tile([128, 512], F32, tag="tp", name="tp")
                for j in range(4):
                    nc.tensor.transpose(
                        tp[:, j * 128:(j + 1) * 128],
                        xins[j][:, t * 128:(t + 1) * 128], identity)
                ch = xt_pool.tile([128, 512], F16, tag="xch", name="xch")
                nc.vector.tensor_copy(ch, tp)
                chunks.append(ch)
            return chunks

        def project_chunk(chunks, w_hf, b_sb, outs):
            for g in range(2):
                ps = ppsum.tile([128, 512], F32, tag="proj", name="ps")
                for t in range(2):
                    nc.tensor.matmul(
                        ps, w_hf[t][:, g * 128:(g + 1) * 128], chunks[t],
                        start=(t == 0), stop=(t == 1))
                nc.vector.tensor_scalar_add(outs[g], ps, b_sb[:, g:g + 1])

        def value_tile(s):
            vin = load_tile(v_d, s, True)
            vtp = tpsum.tile([128, 256], F32, tag="tp", name="vtp")
            for t in range(2):
                nc.tensor.transpose(
                    vtp[:, t * 128:(t + 1) * 128],
                    vin[:, t * 128:(t + 1) * 128], identity)
            vT = vt_pool.tile([128, 256], F16, tag="vT", name="vT")
            nc.vector.tensor_copy(vT, vtp)
            vps = ppsum.tile([128, 256], F32, tag="proj", name="vps")
            for t in range(2):
                nc.tensor.matmul(
                    vps, vT[:, t * 128:(t + 1) * 128], wv_hf[t],
                    start=(t == 0), stop=(t == 1))
            dst = vaug[s].rearrange("p (h e) -> p h e", e=33)[:, :, 0:DH]
            src = vps.rearrange("p (h e) -> p h e", e=DH)
            nc.vector.tensor_copy(dst, src)

        # ---------------- phase A emission (query, then k/v interleaved) ---
        for sg in range(SQ // 512):
            chunks = transpose_group(q_d, sg, False)
            project_chunk(chunks, wq_hf, bq_sb,
                          [qT_sb[g][:, sg * 512:(sg + 1) * 512]
                           for g in range(2)])
        for cch in range(skc // 512):
            chunks = transpose_group(k_d, cch, True)
            project_chunk(chunks, wk_hf, bk_sb,
                          [kT_ch[g][cch] for g in range(2)])
            for s in range(cch * 4, cch * 4 + 4):
                value_tile(s)

        # ---------------- phase B/C: attention ----------------
        for qc in range(SQ // 512):
            wtns = []
            for g in range(2):
                wts = [wt_pool.tile([128, 512], F32, tag="wt", name=f"wt{jj}")
                       for jj in range(2)]
                for kt in range(nkc):
                    for jj in range(2):
                        st = st_pool.tile([128, 1024], F32, tag="st", name="st")
                        for j2 in range(2):
                            j = 2 * jj + j2
                            nc.tensor.matmul(
                                st[:, j2 * 512:(j2 + 1) * 512],
                                kT_ch[g][kt // 4][32 * j:32 * j + 32,
                                                  (kt % 4) * 128:
                                                  (kt % 4) * 128 + 128],
                                qT_sb[g][32 * j:32 * j + 32,
                                         qc * 512:(qc + 1) * 512],
                                start=True, stop=True,
                                tile_position=(32 * j, 0))
                        ptile = pt_pool.tile([128, 1024], F16, tag="pt",
                                             name="ptile")
                        nc.scalar.activation(
                            ptile, st, AF.Exp,
                            bias=maskbias[:, kt:kt + 1], scale=SCALE)
                        for j2 in range(2):
                            h = 4 * g + 2 * jj + j2
                            nc.tensor.matmul(
                                wts[jj][64 * j2:64 * j2 + 33, :],
                                vaug[kt][:, 33 * h:33 * h + 33],
                                ptile[:, j2 * 512:(j2 + 1) * 512],
                                start=(kt == 0), stop=(kt == nkc - 1),
                                tile_position=(0, 64 * j2),
                                skip_group_check=True)

                # evict weightedT to SBUF right away (frees the wt slots),
                # then normalize off the critical path
                for jj in range(2):
                    wcop = osb_pool.tile([128, 512], F32, tag="wcop",
                                         name="wcop")
                    nc.vector.tensor_copy(wcop, wts[jj])
                    rc = rc_pool.tile([128, 512], F32, tag="rc", name="rc")
                    nc.vector.reciprocal(rc[32:33, :], wcop[32:33, :])
                    nc.vector.reciprocal(rc[96:97, :], wcop[96:97, :])
                    bc = st_pool.tile([128, 512], F32, tag="st", name="bc")
                    nc.tensor.matmul(bc[0:32, :], ones_sb[32:33, 0:32],
                                     rc[32:33, :], start=True, stop=True,
                                     tile_position=(32, 0))
                    nc.tensor.matmul(bc[64:96, :], ones_sb[96:97, 0:32],
                                     rc[96:97, :], start=True, stop=True,
                                     tile_position=(96, 64))
                    wtn = wtn_pool.tile([128, 512], F16, tag="wtn", name="wtn")
                    nc.vector.memset(wtn, 0.0)
                    nc.vector.tensor_mul(wtn[0:32, :], wcop[0:32, :],
                                         bc[0:32, :])
                    nc.vector.tensor_mul(wtn[64:96, :], wcop[64:96, :],
                                         bc[64:96, :])
                    wtns.append(wtn)

            for m in range(4):
                ops = wt_pool.tile([128, 256], F32, tag="wt", name="ops")
                for p in range(4):
                    nc.tensor.matmul(
                        ops, wtns[p][:, m * 128:(m + 1) * 128],
                        wo_hf[:, p * 256:(p + 1) * 256],
                        start=(p == 0), stop=False,
                        skip_group_check=True)
                nc.tensor.matmul(ops, ones_hf[0:1, :], bo_hf,
                                 start=False, stop=True,
                                 skip_group_check=True)
                ob = osb_pool.tile([128, 256], F32, tag="ob", name="ob")
                nc.vector.tensor_copy(ob, ops)
                nc.sync.dma_start(
                    out_d[qc * 512 + m * 128:qc * 512 + (m + 1) * 128, :],
                    ob)

    nc.finalize()
    return nc


def _get_nc(nkc):
    key = ("nc", nkc)
    if key not in _CACHE:
        _CACHE[key] = _build_nc(nkc)
    return _CACHE[key]


def kernel(query, key, value, mask, Wq, bq, Wk, bk, Wv, bv, Wo, bo,
           _trace=False):
    query = np.asarray(query, np.float32)
    key = np.asarray(key, np.float32)
    value = np.asarray(value, np.float32)
    mask = np.asarray(mask, np.int32)
    Wq = np.ascontiguousarray(np.asarray(Wq, np.float32))
    Wk = np.ascontiguousarray(np.asarray(Wk, np.float32))
    Wv = np.ascontiguousarray(np.asarray(Wv, np.float32))
    Wo = np.ascontiguousarray(np.asarray(Wo, np.float32))
    bq = np.asarray(bq, np.float32)
    bk = np.asarray(bk, np.float32)
    bv = np.asarray(bv, np.float32)
    bo = np.asarray(bo, np.float32)

    # mask compaction: indices of surviving keys per batch, padded to a
    # multiple of 512 with a duplicate (masked-out) index
    idxs = [np.nonzero(mask[b, 0])[0].astype(np.int32) for b in range(2)]
    nk_max = max(len(ix) for ix in idxs)
    nk_max = max(nk_max, 1)
    skc = ((nk_max + 511) // 512) * 512
    nkc = skc // 128

    nc = _get_nc(nkc)

    wo_arr = np.zeros((128, 4, 256), np.float32)
    for p in range(4):
        wo_arr[0:32, p] = Wo[64 * p:64 * p + 32]
        wo_arr[64:96, p] = Wo[64 * p + 32:64 * p + 64]
    wo_arr = np.ascontiguousarray(wo_arr.reshape(128, 1024))
    bq2 = np.ascontiguousarray(bq.reshape(2, 128).T)
    bk2 = np.ascontiguousarray(bk.reshape(2, 128).T)
    bo2 = np.ascontiguousarray((bv @ Wo + bo).reshape(1, 256))

    in_maps = []
    for c in range(8):
        b, qi = divmod(c, 4)
        ix = idxs[b]
        nk = len(ix)
        ix_pad = np.concatenate(
            [ix, np.full(skc - nk, ix[0] if nk else 0, np.int32)])
        mcomp = (np.arange(skc) < nk).astype(np.int32)
        mb = np.ascontiguousarray(mcomp.reshape(nkc, 128).T)
        ib = np.ascontiguousarray(ix_pad.reshape(nkc, 128).T)
        in_maps.append({
            "q_in": np.ascontiguousarray(query[b, qi * SQ:(qi + 1) * SQ]),
            "k_in": np.ascontiguousarray(key[b]),
            "v_in": np.ascontiguousarray(value[b]),
            "mask_in": mb,
            "idx_in": ib,
            "wq": Wq, "wk": Wk, "wv": Wv, "wo_arr": wo_arr,
            "bq2": bq2, "bk2": bk2, "bo2": bo2,
        })

    res = run_bass_kernel_spmd(nc, in_maps, core_ids=list(range(8)),
                               trace=_trace)
    if _trace:
        _CACHE["last_result"] = res

    out = np.empty((2, 4096, HID), np.float32)
    for c in range(8):
        b, qi = divmod(c, 4)
        out[b, qi * SQ:(qi + 1) * SQ] = res.results[c]["out"]
    return out
